# revision 9
# baseline (speedup 1.0000x reference)
"""2-layer GCN (COO SpMM x2) on 8 Trainium2 NeuronCores.

Strategy (per core, dest-row sharding), v3:
  - Nodes padded to 100352 = 8*98*128. Core c owns 12544 dest rows (98 blocks
    of 128). Sources split into 4 banks of 25088 rows (int16-indexable).
  - Edges routed to the core owning their dest row, grouped by
    (dest block, source bank); each (blk, bank) cell padded to G_BB*128
    tokens (G_BB from data max) so all cores share one compiled module.
  - Sources stored bf16 padded to 256B rows ([*, 128] bf16); dma_gather
    (one SWDGE queue per bank, 4 concurrent) pulls rows directly in bf16.
  - DVE builds unscaled one-hot S tiles with is_equal over [128,jn,64,2]
    broadcast APs (pair-duplicated roff to enable the 2x packed mode); ACT
    applies the per-token val scale to G (token==partition, scale is a
    per-partition vector), writing into the unused upper 64 columns of the
    gather tile; PE accumulates psum += S^T @ G over the 4*G_BB groups of
    each dest block in bf16.
  - Layer 1 results land in an SBUF accumulator; e1 is published to DRAM
    (fp32 out, bf16-padded bounce) and AllGathered; layer 2 repeats the
    schedule reading from gathered e1.
  - Outputs per core: e1, e2, summed = x_shard + e1 + e2. e0 is the input.
"""
import os
import sys

sys.path.insert(0, "/opt/trn_rl_repo")

import numpy as np

N = 100001
NP = 100352          # padded nodes = 8 * 98 * 128
D = 64
CORES = 8
R_C = NP // CORES    # 12544 dest rows per core
NBLK = R_C // 128    # 98 dest blocks per core
BANKS = 4
BANK_R = NP // BANKS  # 25088 source rows per bank

LAST_EXEC_NS = None

_NC_CACHE = {}


def _build_module(G_BB):
    import concourse.bacc as bacc
    import concourse.mybir as mybir
    import concourse.tile as tile

    FP32, BF16, I16 = mybir.dt.float32, mybir.dt.bfloat16, mybir.dt.int16
    Copy = mybir.ActivationFunctionType.Copy

    CAP = G_BB * 128
    T_BANK = NBLK * CAP           # tokens per bank per layer
    G_TOT = NBLK * BANKS * G_BB   # groups per layer
    BPB = int(os.environ.get("KBPB", "2"))   # blocks per gather round
    SP = os.environ.get("KSP", "0") == "1"   # single_packet
    skip_ag = os.environ.get("KSKIP_AG") == "1"

    nc = bacc.Bacc("TRN2", target_bir_lowering=False, debug=False,
                   num_swdge_queues=4)
    xb = nc.dram_tensor("xb", [NP, 128], BF16, kind="ExternalInput")
    idx = nc.dram_tensor("idx", [BANKS, 128, T_BANK // 16], I16,
                         kind="ExternalInput")
    roffd = nc.dram_tensor("roffd", [128, G_TOT, 2], BF16,
                           kind="ExternalInput")
    vald = nc.dram_tensor("vald", [128, G_TOT, 2], BF16,
                          kind="ExternalInput")
    iota = nc.dram_tensor("iota", [128, 128], BF16, kind="ExternalInput")
    x_shard = nc.dram_tensor("x_shard", [R_C, D], FP32, kind="ExternalInput")
    e1_out = nc.dram_tensor("e1_out", [R_C, D], FP32, kind="ExternalOutput")
    e2_out = nc.dram_tensor("e2_out", [R_C, D], FP32, kind="ExternalOutput")
    sum_out = nc.dram_tensor("sum_out", [R_C, D], FP32, kind="ExternalOutput")
    e1_bounce = nc.dram_tensor("e1_bounce", [R_C, 128], BF16)
    e1_full = nc.dram_tensor("e1_full", [NP, 128], BF16, addr_space="Shared")

    with tile.TileContext(nc) as tc:
        with tc.tile_pool(name="meta", bufs=1) as meta, \
             tc.tile_pool(name="gp", bufs=int(os.environ.get("KGBUF", "3"))) as gp, \
             tc.tile_pool(name="sp", bufs=int(os.environ.get("KSBUF", "4"))) as sp, \
             tc.tile_pool(name="op", bufs=4) as op, \
             tc.tile_pool(name="pp", bufs=4, space="PSUM") as pp:

            iota_sb = meta.tile([128, 128], BF16)
            nc.sync.dma_start(out=iota_sb[:], in_=iota[:])
            roffd_sb = meta.tile([128, G_TOT, 2], BF16)
            nc.sync.dma_start(out=roffd_sb[:], in_=roffd[:])
            vald_sb = meta.tile([128, G_TOT, 2], BF16)
            nc.sync.dma_start(out=vald_sb[:], in_=vald[:])
            idx_sb = []
            for b in range(BANKS):
                t = meta.tile([128, T_BANK // 16], I16, tag=f"idx{b}")
                nc.sync.dma_start(out=t[:], in_=idx[b, :, :])
                idx_sb.append(t)
            acc1 = meta.tile([128, NBLK, D], FP32)

            io4 = iota_sb[:].rearrange("p (a b) -> p a b", b=2)[:, None, :, :]

            def layer(src, is2):
                for bp in range(0, NBLK, BPB):
                    nb = min(BPB, NBLK - bp)
                    gts = []
                    for b in range(BANKS):
                        gt = gp.tile([128, BPB * G_BB, 128], BF16,
                                     tag=f"g{b}")
                        nc.gpsimd.dma_gather(
                            gt[:, :nb * G_BB, :],
                            src[b * BANK_R:(b + 1) * BANK_R, :],
                            idx_sb[b][:, bp * CAP // 16:(bp + nb) * CAP // 16],
                            nb * CAP, nb * CAP, 128,
                            queue_num=b, single_packet=SP)
                        gts.append(gt)
                    # S tiles for all groups of this round (unscaled one-hot)
                    NG = nb * BANKS * G_BB
                    G0 = bp * BANKS * G_BB
                    SB = 16
                    s_ts = []
                    for j0 in range(0, NG, SB):
                        jn = min(SB, NG - j0)
                        s_t = sp.tile([128, SB, 64, 2], BF16, tag="s")
                        nc.vector.tensor_tensor(
                            out=s_t[:, :jn, :, :],
                            in0=io4.broadcast_to([128, jn, 64, 2]),
                            in1=roffd_sb[:, G0 + j0:G0 + j0 + jn, None, :]
                                .broadcast_to([128, jn, 64, 2]),
                            op=mybir.AluOpType.is_equal)
                        s_ts.append(s_t)
                    # per-token val scale on G (DVE, batched per bank/block):
                    # scaled rows land in the unused upper 64 columns of the
                    # gather tile
                    for b in range(BANKS):
                        for bi in range(nb):
                            g0b = G0 + bi * BANKS * G_BB + b * G_BB
                            ks = slice(bi * G_BB, (bi + 1) * G_BB)
                            nc.vector.tensor_tensor(
                                out=gts[b][:, ks, 64:128].rearrange(
                                    "p k (a pr) -> p k a pr", pr=2),
                                in0=gts[b][:, ks, 0:64].rearrange(
                                    "p k (a pr) -> p k a pr", pr=2),
                                in1=vald_sb[:, g0b:g0b + G_BB, None, :]
                                    .broadcast_to([128, G_BB, 32, 2]),
                                op=mybir.AluOpType.mult)
                    for bi in range(nb):
                        blk = bp + bi
                        psum = pp.tile([128, D], FP32, tag="ps")
                        for b in range(BANKS):
                            for g in range(G_BB):
                                j = (bi * BANKS + b) * G_BB + g
                                s_ap = s_ts[j // SB][:, j % SB] \
                                    .rearrange("p a b -> p (a b)")
                                nc.tensor.matmul(
                                    psum[:], s_ap,
                                    gts[b][:, bi * G_BB + g, 64:128],
                                    start=(b == 0 and g == 0),
                                    stop=(b == BANKS - 1 and g == G_BB - 1))
                        if not is2:
                            nc.scalar.activation(acc1[:, blk, :], psum[:],
                                                 Copy)
                            e1b = op.tile([128, D], BF16, tag="e1b")
                            nc.scalar.activation(e1b[:], psum[:], Copy)
                            nc.sync.dma_start(
                                out=e1_out[blk * 128:(blk + 1) * 128, :],
                                in_=acc1[:, blk, :])
                            nc.sync.dma_start(
                                out=e1_bounce[blk * 128:(blk + 1) * 128, 0:D],
                                in_=e1b[:])
                        else:
                            e2s = op.tile([128, D], FP32, tag="e2")
                            nc.scalar.activation(e2s[:], psum[:], Copy)
                            nc.sync.dma_start(
                                out=e2_out[blk * 128:(blk + 1) * 128, :],
                                in_=e2s[:])
                            xs = op.tile([128, D], FP32, tag="xs")
                            nc.sync.dma_start(
                                out=xs[:],
                                in_=x_shard[blk * 128:(blk + 1) * 128, :])
                            st = op.tile([128, D], FP32, tag="st")
                            nc.vector.tensor_add(st[:], acc1[:, blk, :],
                                                 psum[:])
                            nc.vector.tensor_add(st[:], st[:], xs[:])
                            nc.sync.dma_start(
                                out=sum_out[blk * 128:(blk + 1) * 128, :],
                                in_=st[:])

            layer(xb, False)
            if not skip_ag:
                with tc.tile_critical():
                    cc_sem = nc.alloc_semaphore("cc_sem")
                    nc.gpsimd.collective_compute(
                        "AllGather", mybir.AluOpType.bypass,
                        replica_groups=[list(range(CORES))],
                        ins=[e1_bounce.ap().opt()],
                        outs=[e1_full.ap().opt()],
                    ).then_inc(cc_sem, 1)
                    nc.gpsimd.wait_ge(cc_sem, 1)
            else:
                nc.sync.dma_start(out=e1_full[:R_C, :], in_=e1_bounce[:])
            layer(e1_full, True)
    nc.compile()
    return nc


def kernel(row_idx, col_idx, adj_vals, emb_weight):
    global LAST_EXEC_NS
    from concourse.bass_utils import run_bass_kernel_spmd
    import ml_dtypes

    row = np.asarray(row_idx).astype(np.int64)
    col = np.asarray(col_idx).astype(np.int64)
    vals = np.asarray(adj_vals).astype(np.float32)
    emb = np.asarray(emb_weight).astype(np.float32)

    x_pad = np.zeros((NP, D), np.float32)
    x_pad[:N] = emb
    xb_pad = np.zeros((NP, 128), ml_dtypes.bfloat16)
    xb_pad[:N, :D] = emb.astype(ml_dtypes.bfloat16)

    core = row // R_C
    blk = (row % R_C) >> 7
    roff_e = row & 127
    bank = col // BANK_R
    idx16 = (col - bank * BANK_R).astype(np.int16)

    cell = (core * NBLK + blk) * BANKS + bank    # global cell, blk-major
    ncell = CORES * NBLK * BANKS
    counts = np.bincount(cell, minlength=ncell)
    G_BB = int(np.ceil(counts.max() / 128))
    CAP = G_BB * 128

    order = np.argsort(cell, kind="stable")
    cell_sorted = cell[order]
    starts = np.zeros(ncell, np.int64)
    starts[1:] = np.cumsum(counts)[:-1]
    rank = np.arange(len(order)) - starts[cell_sorted]
    slot = cell_sorted * CAP + rank              # unique slot per edge

    T_CORE = NBLK * BANKS * CAP
    T_BANK = NBLK * CAP
    G_TOT = NBLK * BANKS * G_BB
    idx_all = np.zeros(CORES * T_CORE, np.int16)
    roff_all = np.zeros(CORES * T_CORE, np.float32)
    val_all = np.zeros(CORES * T_CORE, np.float32)
    idx_all[slot] = idx16[order]
    roff_all[slot] = roff_e[order].astype(np.float32)
    val_all[slot] = vals[order]

    iota_np = np.tile(np.arange(128, dtype=np.float32),
                      (128, 1)).astype(ml_dtypes.bfloat16)

    key = (G_BB, os.environ.get("KSKIP_AG") == "1",
           os.environ.get("KBPB", "2"), os.environ.get("KSP", "0"),
           os.environ.get("KGBUF", "3"), os.environ.get("KSBUF", "4"))
    if key not in _NC_CACHE:
        _NC_CACHE[key] = _build_module(G_BB)
    nc = _NC_CACHE[key]

    in_maps = []
    for c in range(CORES):
        sl = slice(c * T_CORE, (c + 1) * T_CORE)
        # [NBLK, BANKS, CAP] -> banks-major [BANKS, T_BANK]
        idx_c = idx_all[sl].reshape(NBLK, BANKS, CAP).transpose(1, 0, 2)
        idx_banks = np.ascontiguousarray(idx_c.reshape(BANKS, T_BANK))
        # wrap-16 + replicate 8x -> [BANKS, 128, T_BANK//16]
        idx_banks = np.stack([
            np.tile(idx_banks[b].reshape(-1, 16).T, (8, 1))
            for b in range(BANKS)])
        roff_c = roff_all[sl].reshape(G_TOT, 128).T        # [128, G_TOT]
        roffd = np.repeat(roff_c[:, :, None], 2, axis=2)   # [128, G_TOT, 2]
        val_c = val_all[sl].reshape(G_TOT, 128).T
        vald = np.repeat(val_c[:, :, None], 2, axis=2)
        im = {
            "xb": xb_pad,
            "x_shard": x_pad[c * R_C:(c + 1) * R_C],
            "idx": idx_banks,
            "roffd": np.ascontiguousarray(roffd).astype(ml_dtypes.bfloat16),
            "vald": np.ascontiguousarray(vald).astype(ml_dtypes.bfloat16),
            "iota": iota_np,
        }
        in_maps.append(im)

    import time as _time
    nrep = int(os.environ.get("KBENCH_REPS", "1"))
    walls = []
    for _ in range(nrep):
        _t0 = _time.time()
        res = run_bass_kernel_spmd(nc, in_maps, core_ids=list(range(CORES)))
        walls.append(int((_time.time() - _t0) * 1e9))
    globals()["RUN_WALLS"] = walls
    LAST_EXEC_NS = res.exec_time_ns

    e1 = np.concatenate([res.results[c]["e1_out"] for c in range(CORES)])[:N]
    e2 = np.concatenate([res.results[c]["e2_out"] for c in range(CORES)])[:N]
    summed = np.concatenate([res.results[c]["sum_out"]
                             for c in range(CORES)])[:N]
    e0 = emb.copy()
    return (summed, e0, e1, e2)


# revision 18
# speedup vs baseline: 1.0533x; 1.0533x over previous
"""2-layer GCN (COO SpMM x2) on 8 Trainium2 NeuronCores.

Strategy (per core, dest-row sharding), v3:
  - Nodes padded to 100352 = 8*98*128. Core c owns 12544 dest rows (98 blocks
    of 128). Sources split into 4 banks of 25088 rows (int16-indexable).
  - Edges routed to the core owning their dest row, grouped by
    (dest block, source bank); each (blk, bank) cell padded to G_BB*128
    tokens (G_BB from data max) so all cores share one compiled module.
  - Sources stored bf16 padded to 256B rows ([*, 128] bf16); dma_gather
    (one SWDGE queue per bank, 4 concurrent) pulls rows directly in bf16.
  - DVE builds unscaled one-hot S tiles with is_equal over [128,jn,64,2]
    broadcast APs (pair-duplicated roff to enable the 2x packed mode); ACT
    applies the per-token val scale to G (token==partition, scale is a
    per-partition vector), writing into the unused upper 64 columns of the
    gather tile; PE accumulates psum += S^T @ G over the 4*G_BB groups of
    each dest block in bf16.
  - Layer 1 results land in an SBUF accumulator; e1 is published to DRAM
    (fp32 out, bf16-padded bounce) and AllGathered; layer 2 repeats the
    schedule reading from gathered e1.
  - Outputs per core: e1, e2, summed = x_shard + e1 + e2. e0 is the input.
"""
import os
import sys

sys.path.insert(0, "/opt/trn_rl_repo")

import numpy as np

N = 100001
NP = 100352          # padded nodes = 8 * 98 * 128
D = 64
CORES = 8
R_C = NP // CORES    # 12544 dest rows per core
NBLK = R_C // 128    # 98 dest blocks per core
BANKS = 4
BANK_R = NP // BANKS  # 25088 source rows per bank

LAST_EXEC_NS = None

_NC_CACHE = {}


def _build_module(G_BB):
    import concourse.bacc as bacc
    import concourse.mybir as mybir
    import concourse.tile as tile

    FP32, BF16, I16 = mybir.dt.float32, mybir.dt.bfloat16, mybir.dt.int16
    Copy = mybir.ActivationFunctionType.Copy

    CAP = G_BB * 128
    T_BANK = NBLK * CAP           # tokens per bank per layer
    G_TOT = NBLK * BANKS * G_BB   # groups per layer
    BPB = int(os.environ.get("KBPB", "4"))   # blocks per gather round
    SP = os.environ.get("KSP", "0") == "1"   # single_packet
    NCHUNK = int(os.environ.get("KCHUNK", "1"))  # AllGather chunks
    CBLK = NBLK // NCHUNK         # blocks per AG chunk
    skip_ag = os.environ.get("KSKIP_AG") == "1"

    nc = bacc.Bacc("TRN2", target_bir_lowering=False, debug=False,
                   num_swdge_queues=4)
    xb = nc.dram_tensor("xb", [NP, 128], BF16, kind="ExternalInput")
    idx = nc.dram_tensor("idx", [BANKS, 128, T_BANK // 16], I16,
                         kind="ExternalInput")
    roffd = nc.dram_tensor("roffd", [128, G_TOT, 2], BF16,
                           kind="ExternalInput")
    vald = nc.dram_tensor("vald", [128, G_TOT, 2], BF16,
                          kind="ExternalInput")
    iota = nc.dram_tensor("iota", [128, 128], BF16, kind="ExternalInput")
    x_shard = nc.dram_tensor("x_shard", [R_C, D], FP32, kind="ExternalInput")
    e1_out = nc.dram_tensor("e1_out", [R_C, D], FP32, kind="ExternalOutput")
    e2_out = nc.dram_tensor("e2_out", [R_C, D], FP32, kind="ExternalOutput")
    sum_out = nc.dram_tensor("sum_out", [R_C, D], FP32, kind="ExternalOutput")

    with tile.TileContext(nc) as tc:
        with tc.tile_pool(name="meta", bufs=1) as meta, \
             tc.tile_pool(name="dram", bufs=1, space="DRAM") as dram, \
             tc.tile_pool(name="ip", bufs=3) as ip, \
             tc.tile_pool(name="gp", bufs=int(os.environ.get("KGBUF", "3"))) as gp, \
             tc.tile_pool(name="sp", bufs=int(os.environ.get("KSBUF", "4"))) as sp, \
             tc.tile_pool(name="op", bufs=4) as op, \
             tc.tile_pool(name="pp", bufs=4, space="PSUM") as pp:

            e1b_t = dram.tile([R_C, 128], BF16)
            e1f_t = dram.tile([NP, 128], BF16, addr_space="Shared")

            iota_sb = meta.tile([128, 128], BF16)
            nc.sync.dma_start(out=iota_sb[:], in_=iota[:])
            roffd_sb = meta.tile([128, G_TOT, 2], BF16)
            nc.sync.dma_start(out=roffd_sb[:], in_=roffd[:])
            vald_sb = meta.tile([128, G_TOT, 2], BF16)
            nc.sync.dma_start(out=vald_sb[:], in_=vald[:])
            acc1 = meta.tile([128, NBLK, D], FP32)

            io4 = iota_sb[:].rearrange("p (a b) -> p a b", b=2)[:, None, :, :]
            ag_ends = [min((c + 1) * CBLK, NBLK) for c in range(NCHUNK - 1)]
            ag_ends.append(NBLK)

            def layer(src, is2):
                for bp in range(0, NBLK, BPB):
                    nb = min(BPB, NBLK - bp)
                    gts = []
                    for b in range(BANKS):
                        ix = ip.tile([128, BPB * CAP // 16], I16,
                                     tag=f"ix{b}")
                        nc.sync.dma_start(
                            out=ix[:, :nb * CAP // 16],
                            in_=idx[b, :, bp * CAP // 16:(bp + nb) * CAP // 16])
                        gt = gp.tile([128, BPB * G_BB, 128], BF16,
                                     tag=f"g{b}")
                        nc.gpsimd.dma_gather(
                            gt[:, :nb * G_BB, :],
                            src[b * BANK_R:(b + 1) * BANK_R, :],
                            ix[:, :nb * CAP // 16],
                            nb * CAP, nb * CAP, 128,
                            queue_num=b, single_packet=SP)
                        gts.append(gt)
                    # S tiles for all groups of this round (unscaled one-hot)
                    NG = nb * BANKS * G_BB
                    G0 = bp * BANKS * G_BB
                    SB = 16
                    s_ts = []
                    for j0 in range(0, NG, SB):
                        jn = min(SB, NG - j0)
                        s_t = sp.tile([128, SB, 64, 2], BF16, tag="s")
                        nc.vector.tensor_tensor(
                            out=s_t[:, :jn, :, :],
                            in0=io4.broadcast_to([128, jn, 64, 2]),
                            in1=roffd_sb[:, G0 + j0:G0 + j0 + jn, None, :]
                                .broadcast_to([128, jn, 64, 2]),
                            op=mybir.AluOpType.is_equal)
                        s_ts.append(s_t)
                    # per-token val scale on G (DVE, batched per bank/block):
                    # scaled rows land in the unused upper 64 columns of the
                    # gather tile
                    for b in range(BANKS):
                        for bi in range(nb):
                            g0b = G0 + bi * BANKS * G_BB + b * G_BB
                            ks = slice(bi * G_BB, (bi + 1) * G_BB)
                            nc.vector.tensor_tensor(
                                out=gts[b][:, ks, 64:128].rearrange(
                                    "p k (a pr) -> p k a pr", pr=2),
                                in0=gts[b][:, ks, 0:64].rearrange(
                                    "p k (a pr) -> p k a pr", pr=2),
                                in1=vald_sb[:, g0b:g0b + G_BB, None, :]
                                    .broadcast_to([128, G_BB, 32, 2]),
                                op=mybir.AluOpType.mult)
                    for bi in range(nb):
                        blk = bp + bi
                        psum = pp.tile([128, D], FP32, tag="ps")
                        for b in range(BANKS):
                            for g in range(G_BB):
                                j = (bi * BANKS + b) * G_BB + g
                                s_ap = s_ts[j // SB][:, j % SB] \
                                    .rearrange("p a b -> p (a b)")
                                nc.tensor.matmul(
                                    psum[:], s_ap,
                                    gts[b][:, bi * G_BB + g, 64:128],
                                    start=(b == 0 and g == 0),
                                    stop=(b == BANKS - 1 and g == G_BB - 1))
                        if not is2:
                            nc.scalar.activation(acc1[:, blk, :], psum[:],
                                                 Copy)
                            e1b = op.tile([128, D], BF16, tag="e1b")
                            nc.scalar.activation(e1b[:], psum[:], Copy)
                            nc.sync.dma_start(
                                out=e1_out[blk * 128:(blk + 1) * 128, :],
                                in_=acc1[:, blk, :])
                            nc.sync.dma_start(
                                out=e1b_t[blk * 128:(blk + 1) * 128, 0:D],
                                in_=e1b[:])
                            if not skip_ag and (blk + 1) in ag_ends:
                                c = ag_ends.index(blk + 1)
                                b0 = ag_ends[c - 1] if c else 0
                                r0, rn = b0 * 128, (blk + 1 - b0) * 128
                                nc.gpsimd.collective_compute(
                                    "AllGather",
                                    mybir.AluOpType.bypass,
                                    replica_groups=[list(range(CORES))],
                                    ins=[e1b_t[r0:r0 + rn, :].opt()],
                                    outs=[e1f_t[:]
                                          .rearrange("(r n) d -> r n d",
                                                     r=CORES)
                                          [:, r0:r0 + rn, :].opt()],
                                )
                        else:
                            e2s = op.tile([128, D], FP32, tag="e2")
                            nc.scalar.activation(e2s[:], psum[:], Copy)
                            nc.sync.dma_start(
                                out=e2_out[blk * 128:(blk + 1) * 128, :],
                                in_=e2s[:])
                            xs = op.tile([128, D], FP32, tag="xs")
                            nc.sync.dma_start(
                                out=xs[:],
                                in_=x_shard[blk * 128:(blk + 1) * 128, :])
                            st = op.tile([128, D], FP32, tag="st")
                            nc.vector.tensor_add(st[:], acc1[:, blk, :],
                                                 psum[:])
                            nc.vector.tensor_add(st[:], st[:], xs[:])
                            nc.sync.dma_start(
                                out=sum_out[blk * 128:(blk + 1) * 128, :],
                                in_=st[:])

            layer(xb, False)
            if skip_ag:
                nc.sync.dma_start(out=e1f_t[:R_C, :], in_=e1b_t[:])
            layer(e1f_t, True)
    nc.compile()
    return nc


def kernel(row_idx, col_idx, adj_vals, emb_weight):
    global LAST_EXEC_NS
    from concourse.bass_utils import run_bass_kernel_spmd
    import ml_dtypes

    row = np.asarray(row_idx).astype(np.int64)
    col = np.asarray(col_idx).astype(np.int64)
    vals = np.asarray(adj_vals).astype(np.float32)
    emb = np.asarray(emb_weight).astype(np.float32)

    x_pad = np.zeros((NP, D), np.float32)
    x_pad[:N] = emb
    xb_pad = np.zeros((NP, 128), ml_dtypes.bfloat16)
    xb_pad[:N, :D] = emb.astype(ml_dtypes.bfloat16)

    core = row // R_C
    blk = (row % R_C) >> 7
    roff_e = row & 127
    bank = col // BANK_R
    idx16 = (col - bank * BANK_R).astype(np.int16)

    cell = (core * NBLK + blk) * BANKS + bank    # global cell, blk-major
    ncell = CORES * NBLK * BANKS
    counts = np.bincount(cell, minlength=ncell)
    G_BB = int(np.ceil(counts.max() / 128))
    CAP = G_BB * 128

    order = np.argsort(cell, kind="stable")
    cell_sorted = cell[order]
    starts = np.zeros(ncell, np.int64)
    starts[1:] = np.cumsum(counts)[:-1]
    rank = np.arange(len(order)) - starts[cell_sorted]
    slot = cell_sorted * CAP + rank              # unique slot per edge

    T_CORE = NBLK * BANKS * CAP
    T_BANK = NBLK * CAP
    G_TOT = NBLK * BANKS * G_BB
    idx_all = np.zeros(CORES * T_CORE, np.int16)
    roff_all = np.zeros(CORES * T_CORE, np.float32)
    val_all = np.zeros(CORES * T_CORE, np.float32)
    idx_all[slot] = idx16[order]
    roff_all[slot] = roff_e[order].astype(np.float32)
    val_all[slot] = vals[order]

    iota_np = np.tile(np.arange(128, dtype=np.float32),
                      (128, 1)).astype(ml_dtypes.bfloat16)

    key = (G_BB, os.environ.get("KSKIP_AG") == "1",
           os.environ.get("KBPB", "4"), os.environ.get("KSP", "0"),
           os.environ.get("KGBUF", "3"), os.environ.get("KSBUF", "4"),
           os.environ.get("KCHUNK", "1"))
    if key not in _NC_CACHE:
        _NC_CACHE[key] = _build_module(G_BB)
    nc = _NC_CACHE[key]

    in_maps = []
    for c in range(CORES):
        sl = slice(c * T_CORE, (c + 1) * T_CORE)
        # [NBLK, BANKS, CAP] -> banks-major [BANKS, T_BANK]
        idx_c = idx_all[sl].reshape(NBLK, BANKS, CAP).transpose(1, 0, 2)
        idx_banks = np.ascontiguousarray(idx_c.reshape(BANKS, T_BANK))
        # wrap-16 + replicate 8x -> [BANKS, 128, T_BANK//16]
        idx_banks = np.stack([
            np.tile(idx_banks[b].reshape(-1, 16).T, (8, 1))
            for b in range(BANKS)])
        roff_c = roff_all[sl].reshape(G_TOT, 128).T        # [128, G_TOT]
        roffd = np.repeat(roff_c[:, :, None], 2, axis=2)   # [128, G_TOT, 2]
        val_c = val_all[sl].reshape(G_TOT, 128).T
        vald = np.repeat(val_c[:, :, None], 2, axis=2)
        im = {
            "xb": xb_pad,
            "x_shard": x_pad[c * R_C:(c + 1) * R_C],
            "idx": idx_banks,
            "roffd": np.ascontiguousarray(roffd).astype(ml_dtypes.bfloat16),
            "vald": np.ascontiguousarray(vald).astype(ml_dtypes.bfloat16),
            "iota": iota_np,
        }
        in_maps.append(im)

    import time as _time
    nrep = int(os.environ.get("KBENCH_REPS", "1"))
    walls = []
    for _ in range(nrep):
        _t0 = _time.time()
        res = run_bass_kernel_spmd(nc, in_maps, core_ids=list(range(CORES)))
        walls.append(int((_time.time() - _t0) * 1e9))
    globals()["RUN_WALLS"] = walls
    LAST_EXEC_NS = res.exec_time_ns

    e1 = np.concatenate([res.results[c]["e1_out"] for c in range(CORES)])[:N]
    e2 = np.concatenate([res.results[c]["e2_out"] for c in range(CORES)])[:N]
    summed = np.concatenate([res.results[c]["sum_out"]
                             for c in range(CORES)])[:N]
    e0 = emb.copy()
    return (summed, e0, e1, e2)
